# revision 1
# baseline (speedup 1.0000x reference)
"""DenseFastGAT forward on 8 Trainium2 NeuronCores (Bass/Tile).

Math (per batch b):
  z  = x @ W.T + bW                                  [N, O]
  ai = z @ wai.T + bai ; aj = z @ waj.T + baj        [N]
  e  = leakyrelu(ai_i + aj_j, 0.2)
  att = softmax_row(where(adj>0, e, -9e15) ++ sink(-1e9))[:, :N]
  out = att @ z

Kernel strategy:
  - ai/aj fold to ai = x @ (W.T @ wai.T) + const: computed on host in f64
    (tiny: 2 x [N,256]@[256,1] per batch), fed as vectors.
  - Sharding: 8 cores = 2 batches x 4 row-slabs of NI=1024 rows each.
    Each core gets the full-batch adjacency TRANSPOSED slab adjsT[j, i_slab]
    (bf16, exact for 0/1), x.T (bf16) to compute z redundantly, ai/aj.
  - No max-subtraction softmax: p_ij = adj_ij * exp(leakyrelu(ai+aj));
    |ai+aj| <= ~29 so exp is safe in fp32; sink contributes exp(-1e9)=0;
    min row degree is 20 so denominators are well away from 0.
  - exp(leakyrelu(v)) = max(exp(v), exp(0.2 v)) (exp monotone). The two
    exponential fields X1=exp(ai+aj), X2=exp(0.2(ai+aj)) are produced per
    j-tile either on ScalarE (Exp activation with fused per-partition bias;
    route A) or on VectorE (rank-1 product of precomputed exp vectors via
    tensor_scalar at 4x bf16; route B) - the split balances the engines.
  - p is built in [j (partition), i (free)] layout so it is the matmul
    stationary operand: out[i_chunk, :] += p[:, chunk].T @ z_aug where
    z_aug = [z | ones]; the ones column yields the softmax denominator as
    output column 256 for free.
  - max + adjacency-mask run as quad-merged (4 j-tiles per instruction)
    VectorE tensor_tensor ops to amortize fixed costs.
"""

import numpy as np
import ml_dtypes

B = 2
N = 4096
IN_F = 256
O = 256
NCORES = 8
SLABS_PER_B = 4
NI = N // SLABS_PER_B        # 1024 rows per core
JT = N // 128                # 32 j-tiles
NQ = JT // 4                 # 8 quads of j-tiles
IC = NI // 128               # 8 output chunks per core
KA = IN_F + 1                # 257 (x augmented with ones row)
ALPHA = 0.2

# jt-level route assignment: True -> route A (ScalarE exp), False -> B.
N_ROUTE_A = 20
ROUTE_A = [(jt * N_ROUTE_A) % JT < N_ROUTE_A for jt in range(JT)]
# z-phase PSUM->SBUF cast copies: which of the 8 quad-copies go on ScalarE.
CAST_ON_ACT = frozenset((1, 3, 5, 7))

_CACHE = {}


def _build():
    import concourse.bacc as bacc
    import concourse.mybir as mybir
    import concourse.tile as tile

    dt = mybir.dt
    AF = mybir.ActivationFunctionType
    ALU = mybir.AluOpType

    nc = bacc.Bacc("TRN2", target_bir_lowering=False, debug=False,
                   num_devices=NCORES)

    adjsT = nc.dram_tensor("adjsT", [N, NI], dt.bfloat16, kind="ExternalInput")
    xT = nc.dram_tensor("xT", [KA, N], dt.bfloat16, kind="ExternalInput")
    wcomb = nc.dram_tensor("wcomb", [KA, O], dt.bfloat16, kind="ExternalInput")
    ai_row = nc.dram_tensor("ai_row", [1, NI], dt.float32, kind="ExternalInput")
    aj_col = nc.dram_tensor("aj_col", [128, JT], dt.float32, kind="ExternalInput")
    out = nc.dram_tensor("out", [NI, O], dt.float32, kind="ExternalOutput")

    adjq_view = adjsT.ap().rearrange("(q k p) i -> q p k i", k=4, p=128)

    with tile.TileContext(nc) as tc:
        from contextlib import ExitStack
        ctx = ExitStack()
        with ctx:
            consts = ctx.enter_context(tc.tile_pool(name="consts", bufs=1))
            adjp = ctx.enter_context(tc.tile_pool(name="adjp", bufs=2))
            xq = ctx.enter_context(tc.tile_pool(name="xq", bufs=2))
            mp = ctx.enter_context(tc.tile_pool(name="mp", bufs=2))
            pp = ctx.enter_context(tc.tile_pool(name="pp", bufs=2))
            outp = ctx.enter_context(tc.tile_pool(name="outp", bufs=2))
            smallp = ctx.enter_context(tc.tile_pool(name="smallp", bufs=4))

            # ---- constants into SBUF ----
            x0 = consts.tile([128, N], dt.bfloat16, tag="x0")
            x1 = consts.tile([128, N], dt.bfloat16, tag="x1")
            x2 = consts.tile([1, N], dt.bfloat16, tag="x2")
            nc.sync.dma_start(out=x0[:], in_=xT[0:128, :])
            nc.sync.dma_start(out=x1[:], in_=xT[128:256, :])
            nc.sync.dma_start(out=x2[:], in_=xT[256:257, :])
            w0 = consts.tile([128, O], dt.bfloat16, tag="w0")
            w1 = consts.tile([128, O], dt.bfloat16, tag="w1")
            w2 = consts.tile([1, O], dt.bfloat16, tag="w2")
            nc.sync.dma_start(out=w0[:], in_=wcomb[0:128, :])
            nc.sync.dma_start(out=w1[:], in_=wcomb[128:256, :])
            nc.sync.dma_start(out=w2[:], in_=wcomb[256:257, :])

            ai_bc = consts.tile([128, NI], dt.float32, tag="ai_bc")
            nc.gpsimd.dma_start(out=ai_bc[:], in_=ai_row.ap().to_broadcast([128, NI]))
            aj_sb = consts.tile([128, JT], dt.float32, tag="aj_sb")
            nc.sync.dma_start(out=aj_sb[:], in_=aj_col[:])
            aj2_sb = consts.tile([128, JT], dt.float32, tag="aj2_sb")
            nc.vector.tensor_scalar_mul(aj2_sb[:], aj_sb[:], ALPHA)

            # exp vectors for route B
            e1_bc = consts.tile([128, NI], dt.bfloat16, tag="e1_bc")
            e2_bc = consts.tile([128, NI], dt.bfloat16, tag="e2_bc")
            nc.scalar.activation(e1_bc[:], ai_bc[:], AF.Exp)
            nc.scalar.activation(e2_bc[:], ai_bc[:], AF.Exp, scale=ALPHA)
            f1_sb = consts.tile([128, JT], dt.float32, tag="f1_sb")
            f2_sb = consts.tile([128, JT], dt.float32, tag="f2_sb")
            nc.scalar.activation(f1_sb[:], aj_sb[:], AF.Exp)
            nc.scalar.activation(f2_sb[:], aj_sb[:], AF.Exp, scale=ALPHA)

            # ---- z phase: z_aug[j, 0:256] = (x @ W.T + bW), col 256 = 1 ----
            z_all = consts.tile([128, JT, O + 1], dt.bfloat16, tag="z_all")
            nc.vector.memset(z_all[:, :, O], 1.0)
            with tc.tile_pool(name="zpsum", bufs=2, space="PSUM") as zpsum:
                for q in range(NQ):
                    zp = zpsum.tile([128, 4 * O], dt.float32, name="zp")
                    for t in range(4):
                        nt = q * 4 + t
                        sl = slice(nt * 128, (nt + 1) * 128)
                        od = zp[:, t * O:(t + 1) * O]
                        nc.tensor.matmul(od, x0[:, sl], w0[:], start=True, stop=False)
                        nc.tensor.matmul(od, x1[:, sl], w1[:], start=False, stop=False)
                        nc.tensor.matmul(od, x2[:, sl], w2[:], start=False, stop=True)
                    zsrc = zp[:].rearrange("p (t o) -> p t o", t=4)
                    zdst = z_all[:, q * 4:(q + 1) * 4, 0:O]
                    if q in CAST_ON_ACT:
                        nc.scalar.copy(zdst, zsrc)
                    else:
                        nc.vector.tensor_copy(zdst, zsrc)

            # ---- main loop over quads of 4 j-tiles ----
            accp = ctx.enter_context(tc.tile_pool(name="accp", bufs=1, space="PSUM"))
            accs = [accp.tile([128, O + 1], dt.float32, tag=f"acc{ic}",
                              name=f"acc{ic}")
                    for ic in range(IC)]

            for q in range(NQ):
                adjt = adjp.tile([128, 4, NI], dt.bfloat16, name="adjt")
                nc.sync.dma_start(out=adjt[:], in_=adjq_view[q])
                xt1 = xq.tile([128, 4, NI], dt.bfloat16, tag="xt1", name="xt1")
                xt2 = xq.tile([128, 4, NI], dt.bfloat16, tag="xt2", name="xt2")
                for k in range(4):
                    jt = q * 4 + k
                    js = slice(jt, jt + 1)
                    if ROUTE_A[jt]:
                        nc.scalar.activation(xt1[:, k, :], ai_bc[:], AF.Exp,
                                             bias=aj_sb[:, js])
                        nc.scalar.activation(xt2[:, k, :], ai_bc[:], AF.Exp,
                                             bias=aj2_sb[:, js], scale=ALPHA)
                    else:
                        nc.vector.tensor_scalar_mul(xt1[:, k, :], e1_bc[:],
                                                    f1_sb[:, js])
                        nc.vector.tensor_scalar_mul(xt2[:, k, :], e2_bc[:],
                                                    f2_sb[:, js])
                m_t = mp.tile([128, 4, NI], dt.bfloat16, name="m_t")
                nc.vector.tensor_tensor(m_t[:], xt1[:], xt2[:], op=ALU.max)
                p_t = pp.tile([128, 4, NI], dt.bfloat16, name="p_t")
                nc.vector.tensor_tensor(p_t[:], m_t[:], adjt[:], op=ALU.mult)

                for k in range(4):
                    jt = q * 4 + k
                    for ic in range(IC):
                        nc.tensor.matmul(
                            accs[ic][:], p_t[:, k, ic * 128:(ic + 1) * 128],
                            z_all[:, jt, :],
                            start=(jt == 0), stop=(jt == JT - 1))

            # ---- normalize + store ----
            for ic in range(IC):
                r_t = smallp.tile([128, 1], dt.float32, tag="r", name="r_t")
                nc.vector.reciprocal(r_t[:], accs[ic][:, O:O + 1])
                o_t = outp.tile([128, O], dt.float32, name="o_t")
                nc.vector.tensor_scalar_mul(o_t[:], accs[ic][:, 0:O], r_t[:])
                nc.sync.dma_start(out=out[ic * 128:(ic + 1) * 128, :], in_=o_t[:])

    nc.compile()
    return nc


def _get_nc():
    if "nc" not in _CACHE:
        _CACHE["nc"] = _build()
    return _CACHE["nc"]


def kernel(x, adjs, W, bW, wai, bai, waj, baj):
    from concourse import bass_utils

    bf16 = ml_dtypes.bfloat16
    x = np.asarray(x, np.float32)
    adjs = np.asarray(adjs, np.float32)
    W = np.asarray(W, np.float32)
    bW = np.asarray(bW, np.float32)
    wai = np.asarray(wai, np.float32)
    bai = np.asarray(bai, np.float32)
    waj = np.asarray(waj, np.float32)
    baj = np.asarray(baj, np.float32)

    # host-folded attention projections (f64 for accuracy)
    u_i = W.astype(np.float64).T @ wai.astype(np.float64).T        # [256,1]
    c_i = float(bW.astype(np.float64) @ wai[0].astype(np.float64)
                + bai.astype(np.float64)[0])
    u_j = W.astype(np.float64).T @ waj.astype(np.float64).T
    c_j = float(bW.astype(np.float64) @ waj[0].astype(np.float64)
                + baj.astype(np.float64)[0])
    ai = ((x.astype(np.float64) @ u_i)[:, :, 0] + c_i).astype(np.float32)  # [B,N]
    aj = ((x.astype(np.float64) @ u_j)[:, :, 0] + c_j).astype(np.float32)

    # per-batch shared inputs
    wc = np.empty((KA, O), bf16)
    wc[:IN_F, :] = W.T.astype(bf16)
    wc[IN_F, :] = bW.astype(bf16)
    xT_b, ajc = [], []
    for b in range(B):
        xa = np.empty((KA, N), bf16)
        xa[:IN_F, :] = x[b].T.astype(bf16)
        xa[IN_F, :] = np.float32(1.0)
        xT_b.append(xa)
        ajc.append(np.ascontiguousarray(aj[b].reshape(JT, 128).T.astype(np.float32)))

    in_maps = []
    for c in range(NCORES):
        b, s = divmod(c, SLABS_PER_B)
        i0 = s * NI
        adjsT_slab = np.ascontiguousarray(adjs[b][i0:i0 + NI, :].T).astype(bf16)
        in_maps.append({
            "adjsT": adjsT_slab,
            "xT": xT_b[b],
            "wcomb": wc,
            "ai_row": ai[b, i0:i0 + NI].reshape(1, NI).astype(np.float32),
            "aj_col": ajc[b],
        })

    nc = _get_nc()
    res = bass_utils.run_bass_kernel_spmd(
        nc, in_maps, core_ids=list(range(NCORES)),
        **_CACHE.get("run_kwargs", {}))
    _CACHE["last_results"] = res

    out = np.empty((B, N, O), np.float32)
    for c in range(NCORES):
        b, s = divmod(c, SLABS_PER_B)
        out[b, s * NI:(s + 1) * NI, :] = res.results[c]["out"]
    return out

